# revision 25
# baseline (speedup 1.0000x reference)
"""GroupedQueryAttention Trainium2 kernel (v4).

Problem: B=2, T=1024, M=2048, D=128, G=4 kv-groups, H=4 heads/group.
Sharding: core c = 4*b + g  (batch x kv-group), no collectives; host sums
the G partial outputs per batch (fp32 upcast).

v4 changes vs v3 (trace-driven; v3 measured 164.5us):
  - passA is k-only (then q0, v, q1-3): the first matmul needs just
    wk + the first half of x chunk 0 (both split-DMA'd), not wk+wq0
    (v3 stalled 9.6us waiting for wq0 mid-passA)
  - rope swap-dependent ops (b-mul, add) on gpsimd: the DVE only runs
    `a` ops and never head-of-line-blocks the PSUM ring that gates the
    next projection (v3 lost ~5us across pass B)
  - all SBUF pools top-level: phase-2 tiles no longer reuse rope-temp
    space, so exp(h0,j0) doesn't wait for rope q3's swap DMAs (4.4us)
  - exp pipelined one (h,j) unit ahead, same depth as the score matmuls
  - phase-3 PSUM ring bufs=4 + r_sb bufs=3: mch2 no longer WARs on a
    DVE eviction stuck behind the h3-hi normalization chain
"""

import sys
import numpy as np

sys.path.insert(0, "/opt/trn_rl_repo")

import concourse.bass as bass  # noqa: E402
import concourse.tile as tile  # noqa: E402
from concourse import bacc, mybir  # noqa: E402
from concourse.bass_utils import run_bass_kernel_spmd  # noqa: E402
from concourse.masks import make_identity  # noqa: E402
from contextlib import ExitStack  # noqa: E402

F32 = mybir.dt.float32
BF16 = mybir.dt.bfloat16
NPBF16 = mybir.dt.np(BF16)

B, T, M, D, G, H = 2, 1024, 2048, 128, 4, 4
KT = M // 128   # 16 contraction tiles
TT = T // 128   # 8 sequence tiles
ROPE_THETA = 10000.0
NEG = -1.0e30
EXP = mybir.ActivationFunctionType.Exp

_CACHE = {}
_MARKS = []


def _mark(nc, label):
    _MARKS.append((label, int(nc.get_next_instruction_name().split("-")[1])))


def _chunks(i0):
    """Column ranges covering [i0, T), split at the 512 PSUM bank edge."""
    if i0 < 512:
        return [(i0, 512), (512, 1024)]
    return [(i0, 1024)]


def _build_program():
    nc = bacc.Bacc("TRN2", target_bir_lowering=False, debug=False, num_devices=8)

    # all inputs host-pretiled: partition dim first, per-partition rows
    # contiguous in DRAM
    x_d = nc.dram_tensor("xt", [128, KT, T], BF16, kind="ExternalInput").ap()
    wq_d = nc.dram_tensor("wq", [128, H, KT, D], BF16, kind="ExternalInput").ap()
    wk_d = nc.dram_tensor("wk", [128, KT, D], BF16, kind="ExternalInput").ap()
    wv_d = nc.dram_tensor("wv", [128, KT, D], BF16, kind="ExternalInput").ap()
    wo_d = nc.dram_tensor("wo", [128, H, M], BF16, kind="ExternalInput").ap()
    cc_d = nc.dram_tensor("cc", [128, T], F32, kind="ExternalInput").ap()
    ss_d = nc.dram_tensor("ss", [128, T], F32, kind="ExternalInput").ap()
    maddT_d = nc.dram_tensor("maddT", [128, 128], BF16, kind="ExternalInput").ap()
    identrR_d = nc.dram_tensor("identrR", [128, 256], BF16, kind="ExternalInput").ap()
    identrL_d = nc.dram_tensor("identrL", [128, 256], BF16, kind="ExternalInput").ap()
    onec_d = nc.dram_tensor("onec", [128, 1], BF16, kind="ExternalInput").ap()
    r_d = nc.dram_tensor("r", [T, M], BF16, kind="ExternalOutput").ap()

    with tile.TileContext(nc) as tc, ExitStack() as ctx:
        persist = ctx.enter_context(tc.tile_pool(name="persist", bufs=1))

        # ---- all DMAs up front (except wo: deferred so it doesn't steal
        # fabric bandwidth from x during the load-critical first 25us) ----
        # scalar (ACT) HWDGE: weights then rope tables, in need order.
        # wk is split so the very first matmul only waits for 8 k-tiles.
        wk_sb = persist.tile([128, KT, D], BF16)
        nc.scalar.dma_start(out=wk_sb[:, 0:8, :], in_=wk_d[:, 0:8, :])
        wq_sb = persist.tile([128, H, KT, D], BF16)
        nc.scalar.dma_start(out=wq_sb[:, 0], in_=wq_d[:, 0])
        nc.scalar.dma_start(out=wk_sb[:, 8:KT, :], in_=wk_d[:, 8:KT, :])
        wv_sb = persist.tile([128, KT, D], BF16)
        nc.scalar.dma_start(out=wv_sb, in_=wv_d)
        cc_sb = persist.tile([128, T], F32)
        nc.scalar.dma_start(out=cc_sb, in_=cc_d)
        ss_sb = persist.tile([128, T], F32)
        nc.scalar.dma_start(out=ss_sb, in_=ss_d)
        for h in range(1, H):
            nc.scalar.dma_start(out=wq_sb[:, h], in_=wq_d[:, h])
        # sync (SP) HWDGE: x in chunks, first one split in half
        x_tiles = []
        for c in range(4):
            xt = persist.tile([128, 4, T], BF16, name=f"x{c}")
            if c == 0:
                nc.sync.dma_start(out=xt[:, 0:2, :], in_=x_d[:, 0:2, :])
                nc.sync.dma_start(out=xt[:, 2:4, :], in_=x_d[:, 2:4, :])
            else:
                nc.sync.dma_start(out=xt, in_=x_d[:, 4 * c:4 * c + 4, :])
            x_tiles.append(xt)
        # gpsimd SWDGE: small non-urgent consts (big descriptors)
        maddT_sb = persist.tile([128, 128], BF16)
        identrR_sb = persist.tile([128, 256], BF16)
        identrL_sb = persist.tile([128, 256], BF16)
        onec_sb = persist.tile([128, 1], BF16)
        wo_sb = persist.tile([128, H, M], BF16)
        for sb, d in ((maddT_sb, maddT_d), (identrR_sb, identrR_d),
                      (identrL_sb, identrL_d), (onec_sb, onec_d)):
            nc.gpsimd.dma_start(out=sb, in_=d)

        def xs(k):
            """x k-tile k as [128, T] slice."""
            return x_tiles[k // 4][:, k % 4, :]

        qT = persist.tile([128, H, T], BF16)      # roped, scaled q heads
        kT = persist.tile([128, T], BF16)         # roped, scaled k
        vsb = persist.tile([128, TT, 128], BF16)  # v[j-block] = [t, d] tiles
        oT = persist.tile([128, H, T], BF16)      # normalized attn outputs

        # all SBUF working pools top-level: no cross-phase WAR on SBUF
        rtmp = ctx.enter_context(tc.tile_pool(name="rope", bufs=2))
        ptpool = ctx.enter_context(tc.tile_pool(name="pt", bufs=3))
        misc = ctx.enter_context(tc.tile_pool(name="amisc", bufs=3))
        rsb_pool = ctx.enter_context(tc.tile_pool(name="rsb", bufs=2))

        ident_sb = persist.tile([128, 128], BF16)
        make_identity(nc, ident_sb)

        # ---------------- Phase 1: projections + rope ----------------
        with ExitStack() as p1:

            def rope(ps, out_ap, c0, n):
                """ps [128, n] fp32 PSUM -> out_ap bf16 (cols c0:c0+n).

                ev(ACT) + a=ps*cc (DVE) free ps; the swap (sync-queue
                SBUF DMA) and its dependents b=swp*ss, out=a+b run on
                gpsimd so they never block the DVE's in-order queue.
                """
                bufs = 1 if n == T else 2  # full-size ropes are sequential
                ev = rtmp.tile([128, n], F32, tag=f"ev{n}", name=f"ev_{c0}_{n}",
                               bufs=bufs)
                nc.scalar.copy(ev, ps)
                a_t = rtmp.tile([128, n], F32, tag=f"ra{n}", bufs=bufs)
                nc.vector.tensor_mul(a_t, ps, cc_sb[:, c0:c0 + n])
                swp = rtmp.tile([128, n], F32, tag=f"swp{n}", bufs=bufs)
                nc.sync.dma_start(out=swp[0:64, :], in_=ev[64:128, :])
                nc.sync.dma_start(out=swp[64:128, :], in_=ev[0:64, :])
                b_t = rtmp.tile([128, n], F32, tag=f"rb{n}", bufs=bufs)
                nc.gpsimd.tensor_mul(b_t, swp, ss_sb[:, c0:c0 + n])
                nc.gpsimd.tensor_add(out_ap, a_t, b_t)

            # pass A: k & q0 interleaved per k-tile — PE consumes x at
            # ~0.29MB/us, under the ~0.35MB/us fabric delivery rate, so
            # the PE never outruns the x stream (k-only consumed at
            # 0.59MB/us and stalled 14us)
            with ExitStack() as pA:
                ppool = pA.enter_context(
                    tc.tile_pool(name="proj_ps", bufs=1, space="PSUM"))
                vpool = pA.enter_context(
                    tc.tile_pool(name="v_ps", bufs=1, space="PSUM"))
                vtpool = pA.enter_context(
                    tc.tile_pool(name="vt_ps", bufs=2, space="PSUM"))
                _mark(nc, "passA")
                ps_k = ppool.tile([128, T], F32, name="ps_k")
                ps_q0 = ppool.tile([128, T], F32, name="ps_q0")
                for k in range(KT):
                    for ps, w in ((ps_k, wk_sb[:, k, :]),
                                  (ps_q0, wq_sb[:, 0, k, :])):
                        for nch in range(2):
                            nc.tensor.matmul(
                                ps[:, nch * 512:(nch + 1) * 512],
                                lhsT=w,
                                rhs=xs(k)[:, nch * 512:(nch + 1) * 512],
                                start=(k == 0), stop=(k == KT - 1),
                            )
                rope(ps_k, kT, 0, T)
                rope(ps_q0, qT[:, 0, :], 0, T)

                # v wide ([d, t], 1024-col streams: 16 weight loads, not
                # 128 — v-direct's per-128-col reloads ran at half speed),
                # then PE-transpose to the [t, d] tiles PV needs
                _mark(nc, "vproj")
                v_ps = vpool.tile([128, T], F32, name="ps_v")
                for k in range(KT):
                    for nch in range(2):
                        nc.tensor.matmul(
                            v_ps[:, nch * 512:(nch + 1) * 512],
                            lhsT=wv_sb[:, k, :],
                            rhs=xs(k)[:, nch * 512:(nch + 1) * 512],
                            start=(k == 0), stop=(k == KT - 1),
                        )
                vT_bf = rtmp.tile([128, T], BF16, tag="vT", bufs=1)
                nc.vector.tensor_copy(out=vT_bf, in_=v_ps)
                for j in range(TT):
                    tp = vtpool.tile([128, 128], BF16, tag="vt", name=f"vt{j}")
                    nc.tensor.transpose(tp, vT_bf[:, j * 128:(j + 1) * 128],
                                        ident_sb)
                    nc.scalar.copy(vsb[:, j, :], tp)

            # pass B: qT1..3, 1-bank half tiles (bufs=3: the next psB never
            # WARs a rope still reading the previous one)
            hpool = p1.enter_context(tc.tile_pool(name="h_ps", bufs=3, space="PSUM"))
            for h in range(1, H):
                _mark(nc, f"qT{h}_proj")
                for nch in range(2):
                    psB = hpool.tile([128, 512], F32, tag="projB",
                                     name=f"ps_q{h}_{nch}")
                    for k in range(KT):
                        nc.tensor.matmul(
                            psB,
                            lhsT=wq_sb[:, h, k, :],
                            rhs=xs(k)[:, nch * 512:(nch + 1) * 512],
                            start=(k == 0), stop=(k == KT - 1),
                        )
                    rope(psB, qT[:, h, nch * 512:(nch + 1) * 512], nch * 512, 512)

        # wo load now, on sync: its config sits behind the rope-swap DMAs
        # in the SP queue so the 2MB transfer really does wait until the
        # fabric is quiet (a gpsimd trigger would run eagerly at ~10us and
        # steal bandwidth from x)
        nc.sync.dma_start(out=wo_sb, in_=wo_d)

        # ---------------- Phase 2: attention ----------------
        with ExitStack() as p2:
            spool = p2.enter_context(tc.tile_pool(name="s_ps", bufs=2, space="PSUM"))
            opool = p2.enter_context(tc.tile_pool(name="o_ps", bufs=2, space="PSUM"))
            dpool = p2.enter_context(tc.tile_pool(name="d_ps", bufs=2, space="PSUM"))

            pairs = [(h, j) for h in range(H) for j in range(TT)]
            s_tiles = {}
            p_tiles = {}

            def emit_s(h, j):
                """Score block + additive causal mask on the diag (all PE)."""
                i0 = j * 128
                s_ps = spool.tile([128, T], F32, tag="s")
                chs = _chunks(i0)
                for ci, (cs, ce) in enumerate(chs):
                    nc.tensor.matmul(
                        s_ps[:, cs:ce],
                        lhsT=kT[:, i0:i0 + 128],
                        rhs=qT[:, h, cs:ce],
                        start=True, stop=(ci > 0),
                    )
                if j in (3, 7):
                    nc.tensor.matmul(
                        s_ps[:, i0 - 128:i0 + 128], lhsT=maddT_sb, rhs=identrL_sb,
                        start=False, stop=True,
                    )
                else:
                    nc.tensor.matmul(
                        s_ps[:, i0:i0 + 256], lhsT=maddT_sb, rhs=identrR_sb,
                        start=False, stop=True,
                    )
                s_tiles[(h, j)] = s_ps

            def emit_exp(h, j):
                i0 = j * 128
                s_ps = s_tiles.pop((h, j))
                p_sb = ptpool.tile([128, T], BF16, tag="pT")
                nc.scalar.activation(out=p_sb[:, i0:T], in_=s_ps[:, i0:T],
                                     func=EXP)
                p_tiles[(h, j)] = p_sb

            den_t = {}   # (h, half) -> den PSUM [1, 512]
            o_t = {}     # (h, half) -> o PSUM [128, 512]
            o_ev = {}    # (h, half) -> unnormalized o in SBUF f32
            bc_t = {}    # (h, half) -> broadcast 1/den SBUF [128, 512] f32

            def get_half(pool, store, h, half, shape, tag):
                if (h, half) not in store:
                    store[(h, half)] = pool.tile(shape, F32, tag=tag,
                                                 name=f"{tag}_{h}_{half}")
                return store[(h, half)]

            def den_done(h, half):
                """den accumulation stopped: evict + reciprocal (DVE) +
                partition-broadcast (gpsimd, idle in phase 2) now; the DVE
                multiply into oT is deferred until norm_apply."""
                den = den_t.pop((h, half))
                den_sb = misc.tile([1, 512], F32, tag="den",
                                   name=f"den_{h}_{half}")
                nc.scalar.copy(den_sb, den)
                dinv = misc.tile([1, 512], F32, tag="dinv")
                scr = misc.tile([1, 512], F32, tag="dscr")
                nc.vector.reciprocal_approx_accurate(out=dinv, in_=den_sb,
                                                     scratch=scr)
                bc = misc.tile([128, 512], F32, tag="bc",
                               name=f"bc_{h}_{half}")
                nc.gpsimd.partition_broadcast(bc, dinv)
                bc_t[(h, half)] = bc

            def norm_apply(h, half):
                """DVE multiply oT = o_ev * bcast(1/den); deps long ready."""
                lo, hi = (0, 512) if half == 0 else (512, 1024)
                nc.vector.tensor_mul(oT[:, h, lo:hi], o_ev.pop((h, half)),
                                     bc_t.pop((h, half)))

            emit_s(*pairs[0])
            emit_s(*pairs[1])
            emit_exp(*pairs[0])
            for idx, (h, j) in enumerate(pairs):
                _mark(nc, f"att_h{h}_j{j}")
                i0 = j * 128
                if idx + 2 < len(pairs):
                    emit_s(*pairs[idx + 2])
                if idx + 1 < len(pairs):
                    emit_exp(*pairs[idx + 1])
                p_sb = p_tiles.pop((h, j))
                for (cs, ce) in _chunks(i0):
                    half = 0 if ce <= 512 else 1
                    off = half * 512
                    den = get_half(dpool, den_t, h, half, [1, 512], "den")
                    o_ps = get_half(opool, o_t, h, half, [128, 512], "o")
                    last = (j == 3) if half == 0 else (j == TT - 1)
                    nc.tensor.matmul(den[:, cs - off:ce - off], lhsT=onec_sb,
                                     rhs=p_sb[:, cs:ce],
                                     start=(j == 0), stop=last)
                    nc.tensor.matmul(o_ps[:, cs - off:ce - off], lhsT=vsb[:, j, :],
                                     rhs=p_sb[:, cs:ce],
                                     start=(j == 0), stop=last)
                    if last:
                        ev = misc.tile([128, 512], F32, tag=f"oev{half}",
                                       name=f"oev_{h}_{half}")
                        if half == 0:
                            nc.scalar.copy(ev, o_t.pop((h, half)))
                        else:
                            # DVE: runs while ACT does exp(h+1,j0), so the
                            # next head's o-hi WAR clears without a bubble
                            nc.vector.tensor_copy(out=ev,
                                                  in_=o_t.pop((h, half)))
                        o_ev[(h, half)] = ev
                        den_done(h, half)
                # deferred multiplies: deps (reciprocal+broadcast) ready
                if j == 6:
                    norm_apply(h, 0)
                if j == 2 and h > 0:
                    norm_apply(h - 1, 1)
            norm_apply(H - 1, 1)

        # ---------------- Phase 3: output projection ----------------
        with ExitStack() as p3:
            rpool = p3.enter_context(tc.tile_pool(name="r_ps", bufs=4, space="PSUM"))
            for t in range(TT):
                _mark(nc, f"rproj_t{t}")
                r_sb = rsb_pool.tile([128, M], BF16, tag="rsb")
                for mch in range(4):
                    r_ps = rpool.tile([128, 512], F32, tag="r")
                    for h in range(H):
                        nc.tensor.matmul(
                            r_ps,
                            lhsT=oT[:, h, t * 128:(t + 1) * 128],
                            rhs=wo_sb[:, h, mch * 512:(mch + 1) * 512],
                            start=(h == 0), stop=(h == H - 1),
                        )
                    if mch % 2 == 0:
                        nc.vector.tensor_copy(
                            out=r_sb[:, mch * 512:(mch + 1) * 512], in_=r_ps)
                    else:
                        nc.scalar.copy(r_sb[:, mch * 512:(mch + 1) * 512], r_ps)
                    if t == TT - 1:
                        # last block: write each 512-col slab as soon as its
                        # eviction lands, so the final DMA is 4x smaller
                        eng = nc.sync if mch % 2 == 0 else nc.scalar
                        eng.dma_start(
                            out=r_d[t * 128:(t + 1) * 128,
                                    mch * 512:(mch + 1) * 512],
                            in_=r_sb[:, mch * 512:(mch + 1) * 512])
                if t < TT - 1:
                    eng = nc.sync if t % 2 == 0 else nc.scalar
                    eng.dma_start(out=r_d[t * 128:(t + 1) * 128, :], in_=r_sb)

    nc.compile()
    return nc


def _host_tables():
    half = D // 2
    qk = np.float32(D ** (-0.25))
    pos = np.arange(T, dtype=np.float32)[:, None]
    freqs = np.power(np.float32(ROPE_THETA),
                     -np.arange(half, dtype=np.float32) / np.float32(half))[None, :]
    ang = pos * freqs                      # [T, 64] fp32
    cosT = (np.cos(ang) * qk).astype(np.float32).T.copy()   # [64, T]
    sinT = (np.sin(ang) * qk).astype(np.float32).T.copy()
    cc = np.concatenate([cosT, cosT], axis=0)               # [128, T]
    ss = np.concatenate([-sinT, sinT], axis=0)
    # additive causal mask, pre-transposed for lhsT (out = lhsT.T @ I)
    ii = np.arange(128)
    maddT = np.where(ii[None, :] > ii[:, None], np.float32(NEG),
                     np.float32(0.0)).astype(NPBF16)         # [i, j] upper strict
    eye = np.eye(128, dtype=NPBF16)
    zer = np.zeros((128, 128), dtype=NPBF16)
    identrR = np.concatenate([eye, zer], axis=1)   # [I|0]
    identrL = np.concatenate([zer, eye], axis=1)   # [0|I]
    return cc, ss, maddT, identrR, identrL


LAST_RESULTS = None


def kernel(x, w_aq, w_ak, w_av, w_ao, _trace=False, _tmpdir=None):
    global LAST_RESULTS
    if "nc" not in _CACHE:
        _CACHE["nc"] = _build_program()
    nc = _CACHE["nc"]

    cc, ss, maddT, identrR, identrL = _host_tables()
    onec = np.ones((128, 1), dtype=NPBF16)

    # host pre-tiling: partition dim first, per-partition rows contiguous
    xt = [np.ascontiguousarray(
        x[b].T.reshape(KT, 128, T).transpose(1, 0, 2)).astype(NPBF16)
        for b in range(B)]
    wq_t = [np.ascontiguousarray(
        w_aq[g].reshape(H, KT, 128, D).transpose(2, 0, 1, 3)).astype(NPBF16)
        for g in range(G)]
    wk_t = [np.ascontiguousarray(
        w_ak[g].reshape(KT, 128, D).transpose(1, 0, 2)).astype(NPBF16)
        for g in range(G)]
    wv_t = [np.ascontiguousarray(
        w_av[g].reshape(KT, 128, D).transpose(1, 0, 2)).astype(NPBF16)
        for g in range(G)]
    wo_t = [np.ascontiguousarray(
        w_ao[g].transpose(1, 0, 2)).astype(NPBF16) for g in range(G)]

    in_maps = []
    for c in range(8):
        b, g = divmod(c, G)
        in_maps.append({
            "xt": xt[b],
            "wq": wq_t[g], "wk": wk_t[g], "wv": wv_t[g], "wo": wo_t[g],
            "cc": cc, "ss": ss, "maddT": maddT,
            "identrR": identrR, "identrL": identrL,
            "onec": onec,
        })

    res = run_bass_kernel_spmd(nc, in_maps, core_ids=list(range(8)), trace=_trace,
                               tmpdir=_tmpdir)
    LAST_RESULTS = res

    out = np.empty((B, T, M), dtype=np.float32)
    for b in range(B):
        acc = res.results[4 * b]["r"].astype(np.float32)
        for g in range(1, G):
            acc = acc + res.results[4 * b + g]["r"].astype(np.float32)
        out[b] = acc
    return out
